# revision 15
# baseline (speedup 1.0000x reference)
"""Trainium2 Bass kernel for nn_LsqNonneg: batched NNLS.

Algorithm: constant-momentum accelerated projected gradient (converges to the
same NNLS KKT point the reference's 200-iteration FISTA approaches):

    AtA = A.T @ A;  L, mu = extreme eigenvalues;  step = 1/L
    W  = I - step*AtA;  beta = (sqrt(L/mu)-1)/(sqrt(L/mu)+1)
    B  = step * A.T @ X
    S_1 = relu(B); S_0 = 0
    for k = 1..K-1:
        S_{k+1} = relu( [(1+beta)W] S_k + [-beta W] S_{k-1} + B )
    return S_K

Both weight matrices are FIXED -> kept in SBUF, no per-iteration weight DMA.
fp32r matmuls round each operand to an 11-bit mantissa (measured); the
deterministic bias from rounding the fixed weights is suppressed by dithering:
n=8 pre-rounded variants per weight whose per-entry mean equals the exact
value, cycled in a balanced shuffled schedule. B is added into PSUM by the
vector engine in exact fp32 (cheaper than an ident@B matmul on the PE, which
runs fp32r at ~2 cycles/column on hardware).

Prologue: X is sent as bf16 (halves the dominant HBM transfer); step*A is an
exact bf16 hi/lo pair, so only X's rounding enters B and it averages out over
the 512-deep contraction.

Device layout (per core, ns=4096 columns): S packed [128, q=1024]; partition
group g holds columns [g*q,(g+1)*q). Weights are block-diagonal diag4 so one
full-array matmul advances all 4 groups. The loop is software-pipelined as 4
independent 256-column streams, each with its own PSUM ring and per-stream
state tiles: while one stream's PSUM waits on the VectorE B-add + ScalarE
relu, the PE runs the other streams' matmuls, keeping the PE array saturated
(and therefore un-throttled by PE_HAM).
"""

import os
import sys

import numpy as np

for _p in ("/opt/trn_rl_repo", "/root/.axon_site/_ro/trn_rl_repo"):
    if os.path.isdir(_p) and _p not in sys.path:
        sys.path.append(_p)

import ml_dtypes
from contextlib import ExitStack

import concourse.bass as bass
import concourse.bacc as bacc
import concourse.tile as tile
from concourse import mybir
from concourse.bass_utils import run_bass_kernel_spmd

M, KD, N_FULL, N_CORES = 512, 32, 32768, 8
ITERS = 64           # total iterations (S_ITERS is returned)
N_DITHER = 8
DITHER_SEED = 1
NSTR = 2             # independent column streams in the loop

F32 = mybir.dt.float32
F32R = mybir.dt.float32r
BF16 = mybir.dt.bfloat16

LAST_RESULTS = None  # BassKernelResults of the most recent run (for test.py)


def build_program(ns: int, iters: int, n_dither: int):
    q = ns // 4          # free extent of the packed [128, q] S layout
    qs = q // NSTR       # columns per stream
    nsl = q // 512       # 512-wide prologue slices
    assert ns % 2048 == 0 and nsl >= 1 and qs % 256 == 0

    nc = bacc.Bacc("TRN2", target_bir_lowering=False)

    x_d = nc.dram_tensor("x", [M, ns], BF16, kind="ExternalInput")
    apad_d = nc.dram_tensor("apad", [2, 4, M, 128], BF16, kind="ExternalInput")
    wd_d = nc.dram_tensor("wd", [n_dither, 2, 128, 128], F32,
                          kind="ExternalInput")
    out_d = nc.dram_tensor("s_out", [KD, ns], F32, kind="ExternalOutput")

    sched = _dither_schedule(iters, n_dither)

    with ExitStack() as ctx:
        tc = ctx.enter_context(tile.TileContext(nc))
        persist = ctx.enter_context(tc.tile_pool(name="persist", bufs=1))
        xpool = ctx.enter_context(tc.tile_pool(name="xstage", bufs=1))
        psum = ctx.enter_context(tc.tile_pool(name="psum", bufs=2,
                                              space="PSUM"))

        # weights go via SWDGE (gpsimd) so both HWDGE rings are free for X;
        # consolidated DMAs.
        w_sb = persist.tile([128, 2 * n_dither * 128], F32R)
        nc.gpsimd.dma_start(
            w_sb[:].rearrange("p (i j m) -> p i j m", i=n_dither, j=2),
            wd_d[:].rearrange("i j p m -> p i j m").bitcast(F32R))

        # (hl, g, c) chunk of the bf16 hi/lo pair of step*A
        apc = persist.tile([128, 32 * 128], BF16)
        nc.gpsimd.dma_start(
            apc[:].rearrange("p (hl g c m) -> p hl g c m", hl=2, g=4, c=4),
            apad_d[:].rearrange("hl g (c p) m -> p hl g c m", p=128))

        b_sb = persist.tile([128, q], F32)
        # per-(generation, stream) state tiles for exact dependency tracking
        s_st = [[persist.tile([128, qs], F32R, name=f"s{i}_{j}")
                 for j in range(NSTR)] for i in range(3)]
        sout = persist.tile([128, q], F32)

        # ---- prologue: B = (step A).T @ X in packed layout; S_1 = relu(B) ----
        # X row-chunk c, column-half h -> groups 2h, 2h+1
        xts = []
        for c in range(4):
            for h in range(2):
                xt = xpool.tile([128, ns // 2], BF16, name=f"xt{c}{h}")
                eng = nc.sync if (2 * c + h) % 2 == 0 else nc.scalar
                eng.dma_start(
                    xt[:],
                    x_d[128 * c:128 * (c + 1),
                        (ns // 2) * h:(ns // 2) * (h + 1)])
                xts.append(xt)
        pbs = [psum.tile([128, 512], F32, name=f"pb{s}", tag=f"pt{s % NSTR}")
               for s in range(nsl)]
        # every matmul writes the full 128-partition slice (zeros outside its
        # group block), so start/stop flags are per-slice across ALL writers.
        n_acc = [0] * nsl
        total_acc = 32
        for c in range(4):
            for h in range(2):
                xt = xts[2 * c + h]
                for g in (2 * h, 2 * h + 1):
                    for hl in range(2):
                        lhs = apc[:, 128 * (16 * hl + 4 * g + c):
                                  128 * (16 * hl + 4 * g + c + 1)]
                        for s in range(nsl):
                            i_acc = n_acc[s]
                            n_acc[s] = i_acc + 1
                            xoff = (g % 2) * q + 512 * s
                            nc.tensor.matmul(
                                pbs[s][:],
                                lhs,
                                xt[:, xoff:xoff + 512],
                                start=(i_acc == 0),
                                stop=(i_acc == total_acc - 1),
                            )
        spp = 512 // qs  # streams per prologue slice
        for s in range(nsl):
            nc.scalar.copy(b_sb[:, 512 * s:512 * (s + 1)], pbs[s][:])
            for jj in range(spp):
                j = spp * s + jj
                src = pbs[s][:, qs * jj:qs * (jj + 1)]
                if j % 2 == 0:
                    nc.vector.tensor_scalar_max(s_st[1][j][:], src, 0.0)
                else:
                    nc.scalar.activation(s_st[1][j][:], src,
                                         mybir.ActivationFunctionType.Relu)

        # ---- loop: k = 1..iters-1 computes S_{k+1} ----
        for k in range(1, iters):
            i = sched[k]
            wa = w_sb[:, 128 * (2 * i):128 * (2 * i + 1)]
            wb = w_sb[:, 128 * (2 * i + 1):128 * (2 * i + 2)]
            last = (k == iters - 1)
            for j in range(NSTR):
                cur = s_st[k % 3][j]
                prev = s_st[(k - 1) % 3][j]
                dest = (sout[:, qs * j:qs * (j + 1)] if last
                        else s_st[(k + 1) % 3][j][:])
                bsl = b_sb[:, qs * j:qs * (j + 1)]
                # full-bank psum tile (512 f32) so ring buffers never share a
                # bank (PE-write + engine-read same bank is fatal); only the
                # first qs columns are used.
                ptb = psum.tile([128, 512], F32, name=f"pt{k}_{j}",
                                tag=f"pt{j}")
                pt = ptb[:, 0:qs]
                nc.tensor.matmul(pt, wa, cur[:],
                                 start=True, stop=(k == 1))
                if k > 1:
                    nc.tensor.matmul(pt, wb, prev[:],
                                     start=False, stop=True)
                nc.vector.tensor_tensor(pt, pt, bsl,
                                        op=mybir.AluOpType.add)
                nc.scalar.activation(dest, pt,
                                     mybir.ActivationFunctionType.Relu)

        for g in range(4):
            nc.sync.dma_start(out_d[:, g * q:(g + 1) * q],
                              sout[32 * g:32 * (g + 1), :])

    nc.finalize()
    return nc


def _dither_schedule(iters, n):
    sched = np.concatenate([np.arange(n)] * (iters // n + 2))[:iters]
    rng = np.random.default_rng(DITHER_SEED)
    rng.shuffle(sched)
    return sched


def _round11(x):
    u = np.ascontiguousarray(np.asarray(x, dtype=np.float32)).view(np.uint32)
    u = ((u + np.uint32(1 << 11)) >> np.uint32(12)) << np.uint32(12)
    return u.view(np.float32).astype(np.float64)


def _bf16(x):
    return np.asarray(x, dtype=np.float32).astype(ml_dtypes.bfloat16)


def _dither_variants(Mx, n):
    """n 11-bit-exact matrices whose per-entry mean ~= Mx."""
    M64 = np.asarray(Mx, dtype=np.float64)
    hi = _round11(M64)
    ulp = 2.0 ** (np.floor(np.log2(np.abs(M64) + 1e-300)) - 11)
    flo = np.where(hi > M64, hi - ulp, hi)
    fhi = flo + ulp
    frac = np.clip((M64 - flo) / ulp, 0, 1)
    cnt = np.rint(frac * n).astype(int)
    return [np.where(i < cnt, fhi, flo).astype(np.float32) for i in range(n)]


def host_prep(A: np.ndarray, n_dither: int):
    A64 = np.asarray(A, dtype=np.float64)
    AtA = A64.T @ A64
    ev = np.linalg.eigvalsh(AtA)
    L, mu = ev[-1], ev[0]
    step = 1.0 / L
    W = np.eye(KD) - step * AtA
    beta = (np.sqrt(L / mu) - 1.0) / (np.sqrt(L / mu) + 1.0)

    was = _dither_variants(((1.0 + beta) * W).T, n_dither)
    wbs = _dither_variants((-beta * W).T, n_dither)
    wd = np.zeros((n_dither, 2, 128, 128), dtype=np.float32)
    for i in range(n_dither):
        for g in range(4):
            blk = slice(32 * g, 32 * (g + 1))
            wd[i, 0][blk, blk] = was[i]
            wd[i, 1][blk, blk] = wbs[i]

    As = (step * A64).astype(np.float32).astype(np.float64)
    As_h = _bf16(As)
    As_l = _bf16(As - As_h.astype(np.float64))
    apad = np.zeros((2, 4, M, 128), dtype=ml_dtypes.bfloat16)
    for g in range(4):
        apad[0, g, :, 32 * g:32 * (g + 1)] = As_h
        apad[1, g, :, 32 * g:32 * (g + 1)] = As_l
    return wd, apad


_PROGRAM_CACHE = {}


def _get_program(ns, iters, n_dither):
    key = (ns, iters, n_dither)
    if key not in _PROGRAM_CACHE:
        _PROGRAM_CACHE[key] = build_program(ns, iters, n_dither)
    return _PROGRAM_CACHE[key]


def kernel(X: np.ndarray, A: np.ndarray) -> np.ndarray:
    global LAST_RESULTS
    X = np.ascontiguousarray(np.asarray(X, dtype=np.float32))
    A = np.ascontiguousarray(np.asarray(A, dtype=np.float32))
    assert X.shape == (M, N_FULL) and A.shape == (M, KD)

    ns = N_FULL // N_CORES
    wd, apad = host_prep(A, N_DITHER)
    nc = _get_program(ns, ITERS, N_DITHER)

    Xb = _bf16(X)
    in_maps = []
    for c in range(N_CORES):
        in_maps.append({
            "x": np.ascontiguousarray(Xb[:, c * ns:(c + 1) * ns]),
            "apad": apad,
            "wd": wd,
        })

    res = run_bass_kernel_spmd(nc, in_maps, core_ids=list(range(N_CORES)))
    LAST_RESULTS = res
    S = np.concatenate([res.results[c]["s_out"] for c in range(N_CORES)], axis=1)
    return np.ascontiguousarray(S.astype(np.float32))


# revision 16
# speedup vs baseline: 1.4531x; 1.4531x over previous
"""Trainium2 Bass kernel for nn_LsqNonneg: batched NNLS.

Algorithm: constant-momentum accelerated projected gradient (converges to the
same NNLS KKT point the reference's 200-iteration FISTA approaches):

    AtA = A.T @ A;  L, mu = extreme eigenvalues;  step = 1/L
    W  = I - step*AtA;  beta = (sqrt(L/mu)-1)/(sqrt(L/mu)+1)
    B  = step * A.T @ X
    S_1 = relu(B); S_0 = 0
    for k = 1..K-1:
        S_{k+1} = relu( [(1+beta)W] S_k + [-beta W] S_{k-1} + B )
    return S_K

Both weight matrices are FIXED -> kept in SBUF, no per-iteration weight DMA.
fp32r matmuls round each operand to an 11-bit mantissa (measured); the
deterministic bias from rounding the fixed weights is suppressed by dithering:
n=8 pre-rounded variants per weight whose per-entry mean equals the exact
value, cycled in a balanced shuffled schedule. B is added into PSUM by the
vector engine in exact fp32 (cheaper than an ident@B matmul on the PE, which
runs fp32r at ~2 cycles/column on hardware).

Prologue: X is sent as bf16 (halves the dominant HBM transfer); step*A is an
exact bf16 hi/lo pair, so only X's rounding enters B and it averages out over
the 512-deep contraction.

Device layout (per core, ns=4096 columns): S packed [128, q=1024]; partition
group g holds columns [g*q,(g+1)*q). Weights are block-diagonal diag4 so one
full-array matmul advances all 4 groups. The loop is software-pipelined as 4
independent 256-column streams, each with its own PSUM ring and per-stream
state tiles: while one stream's PSUM waits on the VectorE B-add + ScalarE
relu, the PE runs the other streams' matmuls, keeping the PE array saturated
(and therefore un-throttled by PE_HAM).
"""

import os
import sys

import numpy as np

for _p in ("/opt/trn_rl_repo", "/root/.axon_site/_ro/trn_rl_repo"):
    if os.path.isdir(_p) and _p not in sys.path:
        sys.path.append(_p)

import ml_dtypes
from contextlib import ExitStack

import concourse.bass as bass
import concourse.bacc as bacc
import concourse.tile as tile
from concourse import mybir
from concourse.bass_utils import run_bass_kernel_spmd

M, KD, N_FULL, N_CORES = 512, 32, 32768, 8
ITERS = 64           # total iterations (S_ITERS is returned)
N_DITHER = 8
DITHER_SEED = 1
NSTR = 4             # independent column streams in the loop

F32 = mybir.dt.float32
F32R = mybir.dt.float32r
BF16 = mybir.dt.bfloat16

LAST_RESULTS = None  # BassKernelResults of the most recent run (for test.py)


def build_program(ns: int, iters: int, n_dither: int):
    q = ns // 4          # free extent of the packed [128, q] S layout
    qs = q // NSTR       # columns per stream
    nsl = q // 512       # 512-wide prologue slices
    assert ns % 2048 == 0 and nsl >= 1 and qs % 256 == 0

    nc = bacc.Bacc("TRN2", target_bir_lowering=False)

    x_d = nc.dram_tensor("x", [M, ns], BF16, kind="ExternalInput")
    apad_d = nc.dram_tensor("apad", [2, 4, M, 128], BF16, kind="ExternalInput")
    wd_d = nc.dram_tensor("wd", [n_dither, 2, 128, 128], F32,
                          kind="ExternalInput")
    out_d = nc.dram_tensor("s_out", [KD, ns], F32, kind="ExternalOutput")

    sched = _dither_schedule(iters, n_dither)

    with ExitStack() as ctx:
        tc = ctx.enter_context(tile.TileContext(nc))
        persist = ctx.enter_context(tc.tile_pool(name="persist", bufs=1))
        xpool = ctx.enter_context(tc.tile_pool(name="xstage", bufs=1))
        psum = ctx.enter_context(tc.tile_pool(name="psum", bufs=2,
                                              space="PSUM"))

        # weights go via SWDGE (gpsimd) so both HWDGE rings are free for X;
        # consolidated DMAs.
        w_sb = persist.tile([128, 2 * n_dither * 128], F32R)
        nc.gpsimd.dma_start(
            w_sb[:].rearrange("p (i j m) -> p i j m", i=n_dither, j=2),
            wd_d[:].rearrange("i j p m -> p i j m").bitcast(F32R))

        # (hl, g, c) chunk of the bf16 hi/lo pair of step*A
        apc = persist.tile([128, 32 * 128], BF16)
        nc.gpsimd.dma_start(
            apc[:].rearrange("p (hl g c m) -> p hl g c m", hl=2, g=4, c=4),
            apad_d[:].rearrange("hl g (c p) m -> p hl g c m", p=128))

        b_sb = persist.tile([128, q], F32)
        # per-(generation, stream) state tiles for exact dependency tracking
        s_st = [[persist.tile([128, qs], F32R, name=f"s{i}_{j}")
                 for j in range(NSTR)] for i in range(3)]
        sout = persist.tile([128, q], F32)

        # ---- prologue: B = (step A).T @ X in packed layout; S_1 = relu(B) ----
        # X row-chunk c, column-half h -> groups 2h, 2h+1
        xts = []
        for c in range(4):
            for h in range(2):
                xt = xpool.tile([128, ns // 2], BF16, name=f"xt{c}{h}")
                eng = nc.sync if (2 * c + h) % 2 == 0 else nc.scalar
                eng.dma_start(
                    xt[:],
                    x_d[128 * c:128 * (c + 1),
                        (ns // 2) * h:(ns // 2) * (h + 1)])
                xts.append(xt)
        pbs = [psum.tile([128, 512], F32, name=f"pb{s}", tag=f"pt{s % NSTR}")
               for s in range(nsl)]
        # every matmul writes the full 128-partition slice (zeros outside its
        # group block), so start/stop flags are per-slice across ALL writers.
        n_acc = [0] * nsl
        total_acc = 32
        for c in range(4):
            for h in range(2):
                xt = xts[2 * c + h]
                for g in (2 * h, 2 * h + 1):
                    for hl in range(2):
                        lhs = apc[:, 128 * (16 * hl + 4 * g + c):
                                  128 * (16 * hl + 4 * g + c + 1)]
                        for s in range(nsl):
                            i_acc = n_acc[s]
                            n_acc[s] = i_acc + 1
                            xoff = (g % 2) * q + 512 * s
                            nc.tensor.matmul(
                                pbs[s][:],
                                lhs,
                                xt[:, xoff:xoff + 512],
                                start=(i_acc == 0),
                                stop=(i_acc == total_acc - 1),
                            )
        spp = 512 // qs  # streams per prologue slice
        for s in range(nsl):
            nc.scalar.copy(b_sb[:, 512 * s:512 * (s + 1)], pbs[s][:])
            for jj in range(spp):
                j = spp * s + jj
                src = pbs[s][:, qs * jj:qs * (jj + 1)]
                if j % 2 == 0:
                    nc.vector.tensor_scalar_max(s_st[1][j][:], src, 0.0)
                else:
                    nc.scalar.activation(s_st[1][j][:], src,
                                         mybir.ActivationFunctionType.Relu)

        # ---- loop: k = 1..iters-1 computes S_{k+1} ----
        for k in range(1, iters):
            i = sched[k]
            wa = w_sb[:, 128 * (2 * i):128 * (2 * i + 1)]
            wb = w_sb[:, 128 * (2 * i + 1):128 * (2 * i + 2)]
            last = (k == iters - 1)
            for j in range(NSTR):
                cur = s_st[k % 3][j]
                prev = s_st[(k - 1) % 3][j]
                dest = (sout[:, qs * j:qs * (j + 1)] if last
                        else s_st[(k + 1) % 3][j][:])
                bsl = b_sb[:, qs * j:qs * (j + 1)]
                # full-bank psum tile (512 f32) so ring buffers never share a
                # bank (PE-write + engine-read same bank is fatal); only the
                # first qs columns are used.
                ptb = psum.tile([128, 512], F32, name=f"pt{k}_{j}",
                                tag=f"pt{j}")
                pt = ptb[:, 0:qs]
                nc.tensor.matmul(pt, wa, cur[:],
                                 start=True, stop=(k == 1))
                if k > 1:
                    nc.tensor.matmul(pt, wb, prev[:],
                                     start=False, stop=True)
                nc.vector.tensor_tensor(pt, pt, bsl,
                                        op=mybir.AluOpType.add)
                nc.scalar.activation(dest, pt,
                                     mybir.ActivationFunctionType.Relu)

        for g in range(4):
            nc.sync.dma_start(out_d[:, g * q:(g + 1) * q],
                              sout[32 * g:32 * (g + 1), :])

    nc.finalize()
    return nc


def _dither_schedule(iters, n):
    sched = np.concatenate([np.arange(n)] * (iters // n + 2))[:iters]
    rng = np.random.default_rng(DITHER_SEED)
    rng.shuffle(sched)
    return sched


def _round11(x):
    u = np.ascontiguousarray(np.asarray(x, dtype=np.float32)).view(np.uint32)
    u = ((u + np.uint32(1 << 11)) >> np.uint32(12)) << np.uint32(12)
    return u.view(np.float32).astype(np.float64)


def _bf16(x):
    return np.asarray(x, dtype=np.float32).astype(ml_dtypes.bfloat16)


def _dither_variants(Mx, n):
    """n 11-bit-exact matrices whose per-entry mean ~= Mx."""
    M64 = np.asarray(Mx, dtype=np.float64)
    hi = _round11(M64)
    ulp = 2.0 ** (np.floor(np.log2(np.abs(M64) + 1e-300)) - 11)
    flo = np.where(hi > M64, hi - ulp, hi)
    fhi = flo + ulp
    frac = np.clip((M64 - flo) / ulp, 0, 1)
    cnt = np.rint(frac * n).astype(int)
    return [np.where(i < cnt, fhi, flo).astype(np.float32) for i in range(n)]


def host_prep(A: np.ndarray, n_dither: int):
    A64 = np.asarray(A, dtype=np.float64)
    AtA = A64.T @ A64
    ev = np.linalg.eigvalsh(AtA)
    L, mu = ev[-1], ev[0]
    step = 1.0 / L
    W = np.eye(KD) - step * AtA
    beta = (np.sqrt(L / mu) - 1.0) / (np.sqrt(L / mu) + 1.0)

    was = _dither_variants(((1.0 + beta) * W).T, n_dither)
    wbs = _dither_variants((-beta * W).T, n_dither)
    wd = np.zeros((n_dither, 2, 128, 128), dtype=np.float32)
    for i in range(n_dither):
        for g in range(4):
            blk = slice(32 * g, 32 * (g + 1))
            wd[i, 0][blk, blk] = was[i]
            wd[i, 1][blk, blk] = wbs[i]

    As = (step * A64).astype(np.float32).astype(np.float64)
    As_h = _bf16(As)
    As_l = _bf16(As - As_h.astype(np.float64))
    apad = np.zeros((2, 4, M, 128), dtype=ml_dtypes.bfloat16)
    for g in range(4):
        apad[0, g, :, 32 * g:32 * (g + 1)] = As_h
        apad[1, g, :, 32 * g:32 * (g + 1)] = As_l
    return wd, apad


_PROGRAM_CACHE = {}


def _get_program(ns, iters, n_dither):
    key = (ns, iters, n_dither)
    if key not in _PROGRAM_CACHE:
        _PROGRAM_CACHE[key] = build_program(ns, iters, n_dither)
    return _PROGRAM_CACHE[key]


def kernel(X: np.ndarray, A: np.ndarray) -> np.ndarray:
    global LAST_RESULTS
    X = np.ascontiguousarray(np.asarray(X, dtype=np.float32))
    A = np.ascontiguousarray(np.asarray(A, dtype=np.float32))
    assert X.shape == (M, N_FULL) and A.shape == (M, KD)

    ns = N_FULL // N_CORES
    wd, apad = host_prep(A, N_DITHER)
    nc = _get_program(ns, ITERS, N_DITHER)

    Xb = _bf16(X)
    in_maps = []
    for c in range(N_CORES):
        in_maps.append({
            "x": np.ascontiguousarray(Xb[:, c * ns:(c + 1) * ns]),
            "apad": apad,
            "wd": wd,
        })

    res = run_bass_kernel_spmd(nc, in_maps, core_ids=list(range(N_CORES)))
    LAST_RESULTS = res
    S = np.concatenate([res.results[c]["s_out"] for c in range(N_CORES)], axis=1)
    return np.ascontiguousarray(S.astype(np.float32))


# revision 22
# speedup vs baseline: 1.4704x; 1.0119x over previous
"""Trainium2 Bass kernel for nn_LsqNonneg: batched NNLS.

Algorithm: constant-momentum accelerated projected gradient (converges to the
same NNLS KKT point the reference's 200-iteration FISTA approaches):

    AtA = A.T @ A;  L, mu = extreme eigenvalues;  step = 1/L
    W  = I - step*AtA;  beta = (sqrt(L/mu)-1)/(sqrt(L/mu)+1)
    B  = step * A.T @ X
    S_1 = relu(B); S_0 = 0
    for k = 1..K-1:
        S_{k+1} = relu( [(1+beta)W] S_k + [-beta W] S_{k-1} + B )
    return S_K

Both weight matrices are FIXED -> kept in SBUF, no per-iteration weight DMA.
fp32r matmuls round each operand to an 11-bit mantissa (measured); the
deterministic bias from rounding the fixed weights is suppressed by dithering:
n=8 pre-rounded variants per weight whose per-entry mean equals the exact
value, cycled in a balanced shuffled schedule. B is added into PSUM by the
vector engine in exact fp32 (cheaper than an ident@B matmul on the PE, which
runs fp32r at ~2 cycles/column on hardware).

Prologue: X is sent as bf16 (halves the dominant HBM transfer); step*A is an
exact bf16 hi/lo pair, so only X's rounding enters B and it averages out over
the 512-deep contraction.

Device layout (per core, ns=4096 columns): S packed [128, q=1024]; partition
group g holds columns [g*q,(g+1)*q). Weights are block-diagonal diag4 so one
full-array matmul advances all 4 groups. The loop is software-pipelined as 4
independent 256-column streams, each with its own PSUM ring and per-stream
state tiles: while one stream's PSUM waits on the VectorE B-add + ScalarE
relu, the PE runs the other streams' matmuls, keeping the PE array saturated
(and therefore un-throttled by PE_HAM).
"""

import os
import sys

import numpy as np

for _p in ("/opt/trn_rl_repo", "/root/.axon_site/_ro/trn_rl_repo"):
    if os.path.isdir(_p) and _p not in sys.path:
        sys.path.append(_p)

import ml_dtypes
from contextlib import ExitStack

import concourse.bass as bass
import concourse.bacc as bacc
import concourse.tile as tile
from concourse import mybir
from concourse.bass_utils import run_bass_kernel_spmd

M, KD, N_FULL, N_CORES = 512, 32, 32768, 8
ITERS = 64           # total iterations (S_ITERS is returned)
N_DITHER = 8
DITHER_SEED = 1
NSTR = 4             # independent column streams in the loop

F32 = mybir.dt.float32
F32R = mybir.dt.float32r
BF16 = mybir.dt.bfloat16

LAST_RESULTS = None  # BassKernelResults of the most recent run (for test.py)


def build_program(ns: int, iters: int, n_dither: int):
    q = ns // 4          # free extent of the packed [128, q] S layout
    qs = q // NSTR       # columns per stream
    nsl = q // 512       # 512-wide prologue slices
    assert ns % 2048 == 0 and nsl >= 1 and qs % 256 == 0

    nc = bacc.Bacc("TRN2", target_bir_lowering=False)

    x_d = nc.dram_tensor("x", [M, ns], BF16, kind="ExternalInput")
    apad_d = nc.dram_tensor("apad", [2, 4, M, 128], BF16, kind="ExternalInput")
    wd_d = nc.dram_tensor("wd", [n_dither, 2, 128, 128], F32,
                          kind="ExternalInput")
    id_d = nc.dram_tensor("ident", [128, 128], F32, kind="ExternalInput")
    out_d = nc.dram_tensor("s_out", [KD, ns], F32, kind="ExternalOutput")

    sched = _dither_schedule(iters, n_dither)

    with ExitStack() as ctx:
        tc = ctx.enter_context(tile.TileContext(nc))
        persist = ctx.enter_context(tc.tile_pool(name="persist", bufs=1))
        xpool = ctx.enter_context(tc.tile_pool(name="xstage", bufs=1))
        psum = ctx.enter_context(tc.tile_pool(name="psum", bufs=2,
                                              space="PSUM"))

        # weights go via SWDGE (gpsimd) so both HWDGE rings are free for X;
        # consolidated DMAs.
        w_sb = persist.tile([128, 2 * n_dither * 128], F32R)
        nc.gpsimd.dma_start(
            w_sb[:].rearrange("p (i j m) -> p i j m", i=n_dither, j=2),
            wd_d[:].rearrange("i j p m -> p i j m").bitcast(F32R))

        # (hl, g, c) chunk of the bf16 hi/lo pair of step*A
        apc = persist.tile([128, 32 * 128], BF16)
        nc.gpsimd.dma_start(
            apc[:].rearrange("p (hl g c m) -> p hl g c m", hl=2, g=4, c=4),
            apad_d[:].rearrange("hl g (c p) m -> p hl g c m", p=128))

        id_sb = persist.tile([128, 128], F32R)
        nc.gpsimd.dma_start(id_sb[:], id_d[:].bitcast(F32R))

        # PE warm-up: PE_HAM keeps the array throttled (0.65-1.2 GHz) until
        # it has seen a few microseconds of sustained matmul activity.  Junk
        # matmuls on already-loaded weight tiles during the X-DMA window
        # release the throttle before the real prologue matmuls arrive.
        warm = psum.tile([128, 512], F32, name="warm", tag="pt3")
        for wi in range(12):
            nc.tensor.matmul(warm[:], w_sb[:, 0:128],
                             w_sb[:, 0:512],
                             start=(wi == 0), stop=(wi == 11))

        b_sb = persist.tile([128, q], F32R)
        # per-(generation, stream) state tiles for exact dependency tracking
        s_st = [[persist.tile([128, qs], F32R, name=f"s{i}_{j}")
                 for j in range(NSTR)] for i in range(3)]
        sout = persist.tile([128, q], F32)

        # ---- prologue: B = (step A).T @ X in packed layout; S_1 = relu(B) ----
        # X row-chunk c, column-half h -> groups 2h, 2h+1
        xts = []
        for c in range(4):
            for h in range(2):
                xt = xpool.tile([128, ns // 2], BF16, name=f"xt{c}{h}")
                eng = nc.sync if (2 * c + h) % 2 == 0 else nc.scalar
                eng.dma_start(
                    xt[:],
                    x_d[128 * c:128 * (c + 1),
                        (ns // 2) * h:(ns // 2) * (h + 1)])
                xts.append(xt)
        pbs = [psum.tile([128, 512], F32, name=f"pb{s}", tag=f"pt{s % NSTR}")
               for s in range(nsl)]
        # every matmul writes the full 128-partition slice (zeros outside its
        # group block), so start/stop flags are per-slice across ALL writers.
        n_acc = [0] * nsl
        total_acc = 32
        for c in range(4):
            for h in range(2):
                xt = xts[2 * c + h]
                for g in (2 * h, 2 * h + 1):
                    for hl in range(2):
                        lhs = apc[:, 128 * (16 * hl + 4 * g + c):
                                  128 * (16 * hl + 4 * g + c + 1)]
                        for s in range(nsl):
                            i_acc = n_acc[s]
                            n_acc[s] = i_acc + 1
                            xoff = (g % 2) * q + 512 * s
                            nc.tensor.matmul(
                                pbs[s][:],
                                lhs,
                                xt[:, xoff:xoff + 512],
                                start=(i_acc == 0),
                                stop=(i_acc == total_acc - 1),
                            )
        spp = 512 // qs  # streams per prologue slice
        for s in range(nsl):
            nc.scalar.copy(b_sb[:, 512 * s:512 * (s + 1)], pbs[s][:])
            for jj in range(spp):
                j = spp * s + jj
                src = pbs[s][:, qs * jj:qs * (jj + 1)]
                if j % 2 == 0:
                    nc.vector.tensor_scalar_max(s_st[1][j][:], src, 0.0)
                else:
                    nc.scalar.activation(s_st[1][j][:], src,
                                         mybir.ActivationFunctionType.Relu)

        # ---- loop: k = 1..iters-1 computes S_{k+1} ----
        for k in range(1, iters):
            i = sched[k]
            wa = w_sb[:, 128 * (2 * i):128 * (2 * i + 1)]
            wb = w_sb[:, 128 * (2 * i + 1):128 * (2 * i + 2)]
            last = (k == iters - 1)
            for j in range(NSTR):
                cur = s_st[k % 3][j]
                prev = s_st[(k - 1) % 3][j]
                dest = (sout[:, qs * j:qs * (j + 1)] if last
                        else s_st[(k + 1) % 3][j][:])
                bsl = b_sb[:, qs * j:qs * (j + 1)]
                # full-bank psum tile (512 f32) so ring buffers never share a
                # bank (PE-write + engine-read same bank is fatal); only the
                # first qs columns are used.
                ptb = psum.tile([128, 512], F32, name=f"pt{k}_{j}",
                                tag=f"pt{j}")
                pt = ptb[:, 0:qs]
                # engine balance: stream 0 takes B via PE (ident@B) and its
                # relu on VectorE; streams 1-3 add B on VectorE and relu on
                # ScalarE.  This keeps PE/DVE/Act all near 2.0us/iter.
                if j == 0:
                    nc.tensor.matmul(pt, id_sb[:],
                                     b_sb[:, 0:qs],
                                     start=True, stop=False)
                    nc.tensor.matmul(pt, wa, cur[:],
                                     start=False, stop=(k == 1))
                    if k > 1:
                        nc.tensor.matmul(pt, wb, prev[:],
                                         start=False, stop=True)
                    nc.vector.tensor_scalar_max(dest, pt, 0.0)
                else:
                    nc.tensor.matmul(pt, wa, cur[:],
                                     start=True, stop=(k == 1))
                    if k > 1:
                        nc.tensor.matmul(pt, wb, prev[:],
                                         start=False, stop=True)
                    nc.vector.tensor_tensor(pt, pt, bsl.bitcast(F32),
                                            op=mybir.AluOpType.add)
                    nc.scalar.activation(dest, pt,
                                         mybir.ActivationFunctionType.Relu)

        for g in range(4):
            nc.sync.dma_start(out_d[:, g * q:(g + 1) * q],
                              sout[32 * g:32 * (g + 1), :])

    nc.finalize()
    return nc


def _dither_schedule(iters, n):
    sched = np.concatenate([np.arange(n)] * (iters // n + 2))[:iters]
    rng = np.random.default_rng(DITHER_SEED)
    rng.shuffle(sched)
    return sched


def _round11(x):
    u = np.ascontiguousarray(np.asarray(x, dtype=np.float32)).view(np.uint32)
    u = ((u + np.uint32(1 << 11)) >> np.uint32(12)) << np.uint32(12)
    return u.view(np.float32).astype(np.float64)


def _bf16(x):
    return np.asarray(x, dtype=np.float32).astype(ml_dtypes.bfloat16)


def _dither_variants(Mx, n):
    """n 11-bit-exact matrices whose per-entry mean ~= Mx."""
    M64 = np.asarray(Mx, dtype=np.float64)
    hi = _round11(M64)
    ulp = 2.0 ** (np.floor(np.log2(np.abs(M64) + 1e-300)) - 11)
    flo = np.where(hi > M64, hi - ulp, hi)
    fhi = flo + ulp
    frac = np.clip((M64 - flo) / ulp, 0, 1)
    cnt = np.rint(frac * n).astype(int)
    return [np.where(i < cnt, fhi, flo).astype(np.float32) for i in range(n)]


def host_prep(A: np.ndarray, n_dither: int):
    A64 = np.asarray(A, dtype=np.float64)
    AtA = A64.T @ A64
    ev = np.linalg.eigvalsh(AtA)
    L, mu = ev[-1], ev[0]
    step = 1.0 / L
    W = np.eye(KD) - step * AtA
    beta = (np.sqrt(L / mu) - 1.0) / (np.sqrt(L / mu) + 1.0)

    was = _dither_variants(((1.0 + beta) * W).T, n_dither)
    wbs = _dither_variants((-beta * W).T, n_dither)
    wd = np.zeros((n_dither, 2, 128, 128), dtype=np.float32)
    for i in range(n_dither):
        for g in range(4):
            blk = slice(32 * g, 32 * (g + 1))
            wd[i, 0][blk, blk] = was[i]
            wd[i, 1][blk, blk] = wbs[i]

    As = (step * A64).astype(np.float32).astype(np.float64)
    As_h = _bf16(As)
    As_l = _bf16(As - As_h.astype(np.float64))
    apad = np.zeros((2, 4, M, 128), dtype=ml_dtypes.bfloat16)
    for g in range(4):
        apad[0, g, :, 32 * g:32 * (g + 1)] = As_h
        apad[1, g, :, 32 * g:32 * (g + 1)] = As_l
    ident = np.eye(128, dtype=np.float32)
    return wd, apad, ident


_PROGRAM_CACHE = {}


def _get_program(ns, iters, n_dither):
    key = (ns, iters, n_dither)
    if key not in _PROGRAM_CACHE:
        _PROGRAM_CACHE[key] = build_program(ns, iters, n_dither)
    return _PROGRAM_CACHE[key]


def kernel(X: np.ndarray, A: np.ndarray) -> np.ndarray:
    global LAST_RESULTS
    X = np.ascontiguousarray(np.asarray(X, dtype=np.float32))
    A = np.ascontiguousarray(np.asarray(A, dtype=np.float32))
    assert X.shape == (M, N_FULL) and A.shape == (M, KD)

    ns = N_FULL // N_CORES
    wd, apad, ident = host_prep(A, N_DITHER)
    nc = _get_program(ns, ITERS, N_DITHER)

    Xb = _bf16(X)
    in_maps = []
    for c in range(N_CORES):
        in_maps.append({
            "x": np.ascontiguousarray(Xb[:, c * ns:(c + 1) * ns]),
            "apad": apad,
            "wd": wd,
            "ident": ident,
        })

    res = run_bass_kernel_spmd(nc, in_maps, core_ids=list(range(N_CORES)))
    LAST_RESULTS = res
    S = np.concatenate([res.results[c]["s_out"] for c in range(N_CORES)], axis=1)
    return np.ascontiguousarray(S.astype(np.float32))


# revision 24
# speedup vs baseline: 1.5143x; 1.0298x over previous
"""Trainium2 Bass kernel for nn_LsqNonneg: batched NNLS.

Algorithm: constant-momentum accelerated projected gradient (converges to the
same NNLS KKT point the reference's 200-iteration FISTA approaches):

    AtA = A.T @ A;  L, mu = extreme eigenvalues;  step = 1/L
    W  = I - step*AtA;  beta = (sqrt(L/mu)-1)/(sqrt(L/mu)+1)
    B  = step * A.T @ X
    S_1 = relu(B); S_0 = 0
    for k = 1..K-1:
        S_{k+1} = relu( [(1+beta)W] S_k + [-beta W] S_{k-1} + B )
    return S_K

Both weight matrices are FIXED -> kept in SBUF, no per-iteration weight DMA.
fp32r matmuls round each operand to an 11-bit mantissa (measured); the
deterministic bias from rounding the fixed weights is suppressed by dithering:
n=8 pre-rounded variants per weight whose per-entry mean equals the exact
value, cycled in a balanced shuffled schedule. B is added into PSUM by the
vector engine in exact fp32 (cheaper than an ident@B matmul on the PE, which
runs fp32r at ~2 cycles/column on hardware).

Prologue: X is sent as bf16 (halves the dominant HBM transfer); step*A is an
exact bf16 hi/lo pair, so only X's rounding enters B and it averages out over
the 512-deep contraction.

Device layout (per core, ns=4096 columns): S packed [128, q=1024]; partition
group g holds columns [g*q,(g+1)*q). Weights are block-diagonal diag4 so one
full-array matmul advances all 4 groups. The loop is software-pipelined as 4
independent 256-column streams, each with its own PSUM ring and per-stream
state tiles: while one stream's PSUM waits on the VectorE B-add + ScalarE
relu, the PE runs the other streams' matmuls, keeping the PE array saturated
(and therefore un-throttled by PE_HAM).
"""

import os
import sys

import numpy as np

for _p in ("/opt/trn_rl_repo", "/root/.axon_site/_ro/trn_rl_repo"):
    if os.path.isdir(_p) and _p not in sys.path:
        sys.path.append(_p)

import ml_dtypes
from contextlib import ExitStack

import concourse.bass as bass
import concourse.bacc as bacc
import concourse.tile as tile
from concourse import mybir
from concourse.bass_utils import run_bass_kernel_spmd

M, KD, N_FULL, N_CORES = 512, 32, 32768, 8
ITERS = 64           # total iterations (S_ITERS is returned)
N_DITHER = 8
DITHER_SEED = 1
NSTR = 4             # independent column streams in the loop

F32 = mybir.dt.float32
F32R = mybir.dt.float32r
BF16 = mybir.dt.bfloat16

LAST_RESULTS = None  # BassKernelResults of the most recent run (for test.py)


def build_program(ns: int, iters: int, n_dither: int):
    q = ns // 4          # free extent of the packed [128, q] S layout
    qs = q // NSTR       # columns per stream
    nsl = q // 512       # 512-wide prologue slices
    assert ns % 2048 == 0 and nsl >= 1 and qs % 256 == 0

    nc = bacc.Bacc("TRN2", target_bir_lowering=False)

    x_d = nc.dram_tensor("x", [M, ns], BF16, kind="ExternalInput")
    apad_d = nc.dram_tensor("apad", [2, 4, M, 128], BF16, kind="ExternalInput")
    wd_d = nc.dram_tensor("wd", [n_dither, 2, 128, 128], F32,
                          kind="ExternalInput")
    id_d = nc.dram_tensor("ident", [128, 128], F32, kind="ExternalInput")
    out_d = nc.dram_tensor("s_out", [KD, ns], F32, kind="ExternalOutput")

    sched = _dither_schedule(iters, n_dither)

    with ExitStack() as ctx:
        tc = ctx.enter_context(tile.TileContext(nc))
        persist = ctx.enter_context(tc.tile_pool(name="persist", bufs=1))
        xpool = ctx.enter_context(tc.tile_pool(name="xstage", bufs=1))
        psum = ctx.enter_context(tc.tile_pool(name="psum", bufs=2,
                                              space="PSUM"))

        # ident goes first on the sync ring so the PE warm-up can start
        # within ~1us; the big weight DMAs ride the scalar ring (SWDGE/gpsimd
        # DMAs turned out to cost ~20us in end-of-program drains).
        id_sb = persist.tile([128, 128], F32R)
        nc.sync.dma_start(id_sb[:], id_d[:].bitcast(F32R))

        w_sb = persist.tile([128, 2 * n_dither * 128], F32R)
        nc.scalar.dma_start(
            w_sb[:].rearrange("p (i j m) -> p i j m", i=n_dither, j=2),
            wd_d[:].rearrange("i j p m -> p i j m").bitcast(F32R))

        # (hl, g, c) chunk of the bf16 hi/lo pair of step*A
        apc = persist.tile([128, 32 * 128], BF16)
        nc.scalar.dma_start(
            apc[:].rearrange("p (hl g c m) -> p hl g c m", hl=2, g=4, c=4),
            apad_d[:].rearrange("hl g (c p) m -> p hl g c m", p=128))

        # PE warm-up: PE_HAM keeps the array throttled (0.65-1.2 GHz) until
        # it has seen a few microseconds of sustained matmul activity.  Junk
        # matmuls on the already-arrived ident tile during the X-DMA window
        # release the throttle before the real prologue matmuls arrive.
        warm = psum.tile([128, 512], F32, name="warm", tag="pt3")
        for wi in range(12):
            nc.tensor.matmul(warm[:], id_sb[:],
                             w_sb[:, 0:512],
                             start=(wi == 0), stop=(wi == 11))

        b_sb = persist.tile([128, q], F32R)
        # per-(generation, stream) state tiles for exact dependency tracking
        s_st = [[persist.tile([128, qs], F32R, name=f"s{i}_{j}")
                 for j in range(NSTR)] for i in range(3)]
        sout = persist.tile([128, q], F32)

        # ---- prologue: B = (step A).T @ X in packed layout; S_1 = relu(B) ----
        # X row-chunk c, column-half h -> groups 2h, 2h+1
        xts = []
        for c in range(4):
            for h in range(2):
                xt = xpool.tile([128, ns // 2], BF16, name=f"xt{c}{h}")
                eng = nc.sync if (2 * c + h) % 2 == 0 else nc.scalar
                eng.dma_start(
                    xt[:],
                    x_d[128 * c:128 * (c + 1),
                        (ns // 2) * h:(ns // 2) * (h + 1)])
                xts.append(xt)
        pbs = [psum.tile([128, 512], F32, name=f"pb{s}", tag=f"pt{s % NSTR}")
               for s in range(nsl)]
        # every matmul writes the full 128-partition slice (zeros outside its
        # group block), so start/stop flags are per-slice across ALL writers.
        n_acc = [0] * nsl
        total_acc = 32
        for c in range(4):
            for h in range(2):
                xt = xts[2 * c + h]
                for g in (2 * h, 2 * h + 1):
                    for hl in range(2):
                        lhs = apc[:, 128 * (16 * hl + 4 * g + c):
                                  128 * (16 * hl + 4 * g + c + 1)]
                        for s in range(nsl):
                            i_acc = n_acc[s]
                            n_acc[s] = i_acc + 1
                            xoff = (g % 2) * q + 512 * s
                            nc.tensor.matmul(
                                pbs[s][:],
                                lhs,
                                xt[:, xoff:xoff + 512],
                                start=(i_acc == 0),
                                stop=(i_acc == total_acc - 1),
                            )
        spp = 512 // qs  # streams per prologue slice
        for s in range(nsl):
            nc.scalar.copy(b_sb[:, 512 * s:512 * (s + 1)], pbs[s][:])
            for jj in range(spp):
                j = spp * s + jj
                src = pbs[s][:, qs * jj:qs * (jj + 1)]
                if j % 2 == 0:
                    nc.vector.tensor_scalar_max(s_st[1][j][:], src, 0.0)
                else:
                    nc.scalar.activation(s_st[1][j][:], src,
                                         mybir.ActivationFunctionType.Relu)

        # ---- loop: k = 1..iters-1 computes S_{k+1} ----
        for k in range(1, iters):
            i = sched[k]
            wa = w_sb[:, 128 * (2 * i):128 * (2 * i + 1)]
            wb = w_sb[:, 128 * (2 * i + 1):128 * (2 * i + 2)]
            last = (k == iters - 1)
            for j in range(NSTR):
                cur = s_st[k % 3][j]
                prev = s_st[(k - 1) % 3][j]
                dest = (sout[:, qs * j:qs * (j + 1)] if last
                        else s_st[(k + 1) % 3][j][:])
                bsl = b_sb[:, qs * j:qs * (j + 1)]
                # full-bank psum tile (512 f32) so ring buffers never share a
                # bank (PE-write + engine-read same bank is fatal); only the
                # first qs columns are used.
                ptb = psum.tile([128, 512], F32, name=f"pt{k}_{j}",
                                tag=f"pt{j}")
                pt = ptb[:, 0:qs]
                # engine balance: stream 0 takes B via PE (ident@B) and its
                # relu on VectorE; streams 1-3 add B on VectorE and relu on
                # ScalarE.  This keeps PE/DVE/Act all near 2.0us/iter.
                if j == 0:
                    nc.tensor.matmul(pt, id_sb[:],
                                     b_sb[:, 0:qs],
                                     start=True, stop=False)
                    nc.tensor.matmul(pt, wa, cur[:],
                                     start=False, stop=(k == 1))
                    if k > 1:
                        nc.tensor.matmul(pt, wb, prev[:],
                                         start=False, stop=True)
                    nc.vector.tensor_scalar_max(dest, pt, 0.0)
                else:
                    nc.tensor.matmul(pt, wa, cur[:],
                                     start=True, stop=(k == 1))
                    if k > 1:
                        nc.tensor.matmul(pt, wb, prev[:],
                                         start=False, stop=True)
                    nc.vector.tensor_tensor(pt, pt, bsl.bitcast(F32),
                                            op=mybir.AluOpType.add)
                    nc.scalar.activation(dest, pt,
                                         mybir.ActivationFunctionType.Relu)

        for g in range(4):
            nc.sync.dma_start(out_d[:, g * q:(g + 1) * q],
                              sout[32 * g:32 * (g + 1), :])

    nc.finalize()
    return nc


def _dither_schedule(iters, n):
    sched = np.concatenate([np.arange(n)] * (iters // n + 2))[:iters]
    rng = np.random.default_rng(DITHER_SEED)
    rng.shuffle(sched)
    return sched


def _round11(x):
    u = np.ascontiguousarray(np.asarray(x, dtype=np.float32)).view(np.uint32)
    u = ((u + np.uint32(1 << 11)) >> np.uint32(12)) << np.uint32(12)
    return u.view(np.float32).astype(np.float64)


def _bf16(x):
    return np.asarray(x, dtype=np.float32).astype(ml_dtypes.bfloat16)


def _dither_variants(Mx, n):
    """n 11-bit-exact matrices whose per-entry mean ~= Mx."""
    M64 = np.asarray(Mx, dtype=np.float64)
    hi = _round11(M64)
    ulp = 2.0 ** (np.floor(np.log2(np.abs(M64) + 1e-300)) - 11)
    flo = np.where(hi > M64, hi - ulp, hi)
    fhi = flo + ulp
    frac = np.clip((M64 - flo) / ulp, 0, 1)
    cnt = np.rint(frac * n).astype(int)
    return [np.where(i < cnt, fhi, flo).astype(np.float32) for i in range(n)]


def host_prep(A: np.ndarray, n_dither: int):
    A64 = np.asarray(A, dtype=np.float64)
    AtA = A64.T @ A64
    ev = np.linalg.eigvalsh(AtA)
    L, mu = ev[-1], ev[0]
    step = 1.0 / L
    W = np.eye(KD) - step * AtA
    beta = (np.sqrt(L / mu) - 1.0) / (np.sqrt(L / mu) + 1.0)

    was = _dither_variants(((1.0 + beta) * W).T, n_dither)
    wbs = _dither_variants((-beta * W).T, n_dither)
    wd = np.zeros((n_dither, 2, 128, 128), dtype=np.float32)
    for i in range(n_dither):
        for g in range(4):
            blk = slice(32 * g, 32 * (g + 1))
            wd[i, 0][blk, blk] = was[i]
            wd[i, 1][blk, blk] = wbs[i]

    As = (step * A64).astype(np.float32).astype(np.float64)
    As_h = _bf16(As)
    As_l = _bf16(As - As_h.astype(np.float64))
    apad = np.zeros((2, 4, M, 128), dtype=ml_dtypes.bfloat16)
    for g in range(4):
        apad[0, g, :, 32 * g:32 * (g + 1)] = As_h
        apad[1, g, :, 32 * g:32 * (g + 1)] = As_l
    ident = np.eye(128, dtype=np.float32)
    return wd, apad, ident


_PROGRAM_CACHE = {}


def _get_program(ns, iters, n_dither):
    key = (ns, iters, n_dither)
    if key not in _PROGRAM_CACHE:
        _PROGRAM_CACHE[key] = build_program(ns, iters, n_dither)
    return _PROGRAM_CACHE[key]


def kernel(X: np.ndarray, A: np.ndarray) -> np.ndarray:
    global LAST_RESULTS
    X = np.ascontiguousarray(np.asarray(X, dtype=np.float32))
    A = np.ascontiguousarray(np.asarray(A, dtype=np.float32))
    assert X.shape == (M, N_FULL) and A.shape == (M, KD)

    ns = N_FULL // N_CORES
    wd, apad, ident = host_prep(A, N_DITHER)
    nc = _get_program(ns, ITERS, N_DITHER)

    Xb = _bf16(X)
    in_maps = []
    for c in range(N_CORES):
        in_maps.append({
            "x": np.ascontiguousarray(Xb[:, c * ns:(c + 1) * ns]),
            "apad": apad,
            "wd": wd,
            "ident": ident,
        })

    res = run_bass_kernel_spmd(nc, in_maps, core_ids=list(range(N_CORES)))
    LAST_RESULTS = res
    S = np.concatenate([res.results[c]["s_out"] for c in range(N_CORES)], axis=1)
    return np.ascontiguousarray(S.astype(np.float32))
